# revision 43
# baseline (speedup 1.0000x reference)
"""BertSelfAttention Trainium2 kernel.

Full inputs in, full output out. Sharding: 8 cores = (batch b in {0,1}) x
(head-group hg in {0..3}); each core computes 4 heads of one batch and
produces the output feature slice out[b, :, hg*256:(hg+1)*256].

Per-core schedule (v2 — ACT-bound software pipeline):
  The exp of the 16.8M scores per core runs on the Scalar (ACT) engine at
  1 elem/cycle/lane @ 1.2 GHz => ~109us floor + per-instruction overhead.
  ACT is therefore the binding engine; the design minimizes ACT
  instructions (80 exps of 1-2k cols via a 4-bank + 3-bank PSUM
  ping-pong) and keeps the PE (123.5us of fp16 matmul cols at 2.4 GHz)
  saturated underneath it:

  - scores are produced side-sequentially per iteration (it = (hp, qb)),
    in k-tile batches [4,3,4,3,2] alternating the two scores PSUM pools;
  - each batch is exp'd in ONE ACTIVATE (bias=-4, scale=1/8) into a
    persistent per-side e-buffer [128, 8192] f16;
  - ctx consumes e per-batch (lag 2) into a shared 1-bank PSUM work tile,
    accumulated across batches in SBUF f32 by the DVE, so the kernel has
    no big serial exp->ctx tail;
  - Q/K/V projections are filler work, scheduled by DMA arrival to keep
    the PE dense from ~7us (HAM clock stays at 8/8 = 2.4 GHz);
  - x is DMA'd as two [8][128][1024] tensors (2KB lines) so the whole
    input load finishes ~18us while projections overlap it.
"""

import numpy as np

B = 2
S = 2048
H = 1024
NH = 16
HD = 64

NCORES = 8
HPC = 4          # heads per core
DS = HPC * HD    # 256 output dims per core
FT = H // 128    # 8 f-tiles (contraction tiles for projections)
KT = S // 128    # 16 key tiles
QB = 4           # q blocks of 512
QBS = 512
VW = HPC * (HD + 1)  # 260: 4 heads x (64 V dims + 1 em column)

EXP_BIAS = -4.0  # uniform shift inside exp; cancels in softmax, guards fp16

# scores k-tile batches per (side, bi): (k0, nk, pool); pool A = 4 banks,
# pool B = 2. Both sides strictly alternate pools (A,B,A,B,...) so no
# batch ever waits on the exp of the batch right before it (the WAR wait
# on the previous same-pool exp is the main PE stall point, and any PE
# gap near 1us re-throttles the HAM clock gate).
BATCH_PAT = [
    [(0, 4, 0), (4, 2, 1), (6, 4, 0), (10, 2, 1), (12, 2, 0), (14, 2, 1)],
    [(0, 4, 0), (4, 2, 1), (6, 4, 0), (10, 2, 1), (12, 2, 0), (14, 2, 1)],
]
NB_BATCH = [6, 6]

_CACHE = {}


def _build_program(split_waits=True):
    import concourse.bass as bass
    import concourse.mybir as mybir
    import concourse.tile as tile
    from concourse.tile_rust import add_dep_helper
    from concourse.vector_clock import ScopedClock

    f32 = mybir.dt.float32
    f16 = mybir.dt.float16
    AF = mybir.ActivationFunctionType
    OP = mybir.AluOpType

    class SplitDrainTileContext(tile.TileContext):
        """The walrus build here rejects instructions with more than one
        sync wait ("Too many sync wait commands"); hoist excess waits onto
        preceding same-engine NOPs."""

        MAX_WAITS_PER_DRAIN = 1
        split_waits_enabled = True

        def _drain_and_barrier(self, tick_clock, wait_clock):
            drain_inst = self.nc.sync.drain()
            wait_clock.add_sem_waits(
                drain_inst.ins, ScopedClock({None: tick_clock.global_clock})
            )
            self.nc.all_engine_barrier()
            assert self.sems is not None
            popped = self.nc._tile_sem_poison_stack.pop()
            assert popped is self._sem_poison
            self.nc.clear_and_free_semaphores(list(self.sems.allocated().values()))
            self.nc.all_engine_barrier()
            if self.split_waits_enabled:
                self._split_multi_waits()

        def _split_multi_waits(self):
            k = self.MAX_WAITS_PER_DRAIN
            nc = self.nc
            for bb in nc.bb_map.values():
                il = bb.bb.instructions
                new = []
                for inst in il:
                    si = getattr(inst, "sync_info", None)
                    waits = list(si.on_wait) if si is not None and si.on_wait else []
                    if len(waits) > k:
                        for j in range(0, len(waits) - k, k):
                            nop = mybir.InstNoOp(
                                name=nc.get_next_instruction_name(),
                                engine=inst.engine,
                                sync_info=mybir.SyncInfo(
                                    on_wait=waits[j : j + k], on_update=[]
                                ),
                                bass_nofuse=True,
                            )
                            new.append(nop)
                        inst.sync_info = mybir.SyncInfo(
                            on_wait=waits[len(waits) - k :],
                            on_update=list(si.on_update) if si.on_update else [],
                        )
                    new.append(inst)
                il[:] = new

    nc = bass.Bass("TRN2", target_bir_lowering=False, debug=False,
                   num_devices=NCORES)

    # DRAM inputs (per-core layouts prepared host-side)
    wk0_d = nc.dram_tensor("wk0", [128, FT * 128], f16, kind="ExternalInput")
    wk1_d = nc.dram_tensor("wk1", [128, FT * 128], f16, kind="ExternalInput")
    wq0_d = nc.dram_tensor("wq0", [128, FT * 128], f16, kind="ExternalInput")
    wq1_d = nc.dram_tensor("wq1", [128, FT * 128], f16, kind="ExternalInput")
    wvA_d = nc.dram_tensor("wvA", [128, FT * VW], f16, kind="ExternalInput")
    xn0_d = nc.dram_tensor("xn0", [FT, 128, 512], f16, kind="ExternalInput")
    xn1_d = nc.dram_tensor("xn1", [FT, 128, 512], f16, kind="ExternalInput")
    xn23_d = nc.dram_tensor("xn23", [FT, 128, 1024], f16, kind="ExternalInput")
    bq_d = nc.dram_tensor("bq", [2, 128, 1], f32, kind="ExternalInput")
    bk_d = nc.dram_tensor("bk", [2, 128, 1], f32, kind="ExternalInput")
    bvb_d = nc.dram_tensor("bvb", [128, DS], f32, kind="ExternalInput")
    em_d = nc.dram_tensor("em", [128, KT], f32, kind="ExternalInput")
    out_d = nc.dram_tensor("out", [S, DS], f32, kind="ExternalOutput")

    SplitDrainTileContext.split_waits_enabled = split_waits
    with SplitDrainTileContext(nc) as tc:
        from contextlib import ExitStack

        with ExitStack() as ctx:
            const = ctx.enter_context(tc.tile_pool(name="const", bufs=1))
            wpool = ctx.enter_context(tc.tile_pool(name="wpool", bufs=1))
            xpool = ctx.enter_context(tc.tile_pool(name="xpool", bufs=1))
            qk = ctx.enter_context(tc.tile_pool(name="qk", bufs=1))
            vp = ctx.enter_context(tc.tile_pool(name="vp", bufs=1))
            epool = ctx.enter_context(tc.tile_pool(name="epool", bufs=1))
            apool = ctx.enter_context(tc.tile_pool(name="apool", bufs=1))
            opool = ctx.enter_context(tc.tile_pool(name="opool", bufs=1))
            rpool = ctx.enter_context(tc.tile_pool(name="rpool", bufs=1))

            # ---- constants / small DMAs ----
            bq_sb = [const.tile([128, 1], f32, tag=f"bq{m}", bufs=1,
                                name=f"bq_sb{m}") for m in range(2)]
            bk_sb = [const.tile([128, 1], f32, tag=f"bk{m}", bufs=1,
                                name=f"bk_sb{m}") for m in range(2)]
            bvb_sb = const.tile([128, DS], f32, tag="bvb", bufs=1, name="bvb_sb")
            em_sb = const.tile([128, KT], f32, tag="em", bufs=1, name="em_sb")
            ebias = const.tile([128, 1], f32, tag="ebias", bufs=1, name="ebias")

            # ---- big persistent SBUF ----
            wm_sb = [[wpool.tile([128, FT * 128], f16, tag=f"w{w}{m}", bufs=1,
                                 name=f"w{w}{m}") for m in range(2)]
                     for w in range(2)]  # [wk, wq] x [m0, m1]
            wv_sb = wpool.tile([128, FT * VW], f16, tag="wv", bufs=1, name="wv")
            x0 = [xpool.tile([128, 512], f16, tag=f"x0_{ft}", bufs=1,
                             name=f"x0_{ft}") for ft in range(FT)]
            x1 = [xpool.tile([128, 512], f16, tag=f"x1_{ft}", bufs=1,
                             name=f"x1_{ft}") for ft in range(FT)]
            x23 = [xpool.tile([128, 1024], f16, tag=f"x23_{ft}", bufs=1,
                              name=f"x23_{ft}") for ft in range(FT)]

            def xt(ft, nb):
                if nb == 0:
                    return x0[ft][:]
                if nb == 1:
                    return x1[ft][:]
                return x23[ft][:, (nb - 2) * 512:(nb - 1) * 512]

            qt = [qk.tile([128, S], f16, tag=f"qt{m}", bufs=1, name=f"qt{m}")
                  for m in range(2)]
            kt_sb = [qk.tile([128, S], f16, tag=f"kt{m}", bufs=1, name=f"kt{m}")
                     for m in range(2)]
            vones = [vp.tile([128, VW], f16, tag=f"v{st}", bufs=1,
                             name=f"vones{st}") for st in range(KT)]
            # persistent per-side exp buffers, one iteration at a time
            esb = [epool.tile([128, KT * QBS], f16, tag=f"e{s}", bufs=1,
                              name=f"e{s}") for s in range(2)]
            # per-side ctx accumulators (f32): 4 qq x (64 ctx + 1 sum)
            acc = [apool.tile([128, 4 * (HD + 1)], f32, tag=f"acc{s}", bufs=1,
                              name=f"acc{s}") for s in range(2)]

            # ---- input DMAs in arrival-priority order ----
            for m in range(2):
                nc.sync.dma_start(bq_sb[m][:], bq_d.ap()[m])
                nc.sync.dma_start(bk_sb[m][:], bk_d.ap()[m])
            nc.sync.dma_start(em_sb[:], em_d.ap())
            # critical path first: wk0/wq0 + x nb0 unblock the first K/Q
            # projections; then m1 weights + x nb1; wv/bvb/x23 last and
            # gated so the early DMAs get full bandwidth.
            nc.sync.dma_start(wm_sb[0][0][:], wk0_d.ap())
            nc.sync.dma_start(wm_sb[1][0][:], wq0_d.ap())

            x0_dmas = [nc.sync.dma_start(x0[ft][:], xn0_d.ap()[ft])
                       for ft in range(FT)]
            tier2 = [nc.sync.dma_start(wm_sb[0][1][:], wk1_d.ap()),
                     nc.sync.dma_start(wm_sb[1][1][:], wq1_d.ap())]
            x1_dmas = [nc.sync.dma_start(x1[ft][:], xn1_d.ap()[ft])
                       for ft in range(FT)]
            for dma in tier2 + x1_dmas:
                add_dep_helper(dma.ins, x0_dmas[-1].ins, sync=True,
                               reason="dma-priority")
            gated_dmas = []
            gated_dmas.append(nc.sync.dma_start(wv_sb[:], wvA_d.ap()))
            gated_dmas.append(nc.sync.dma_start(bvb_sb[:], bvb_d.ap()))
            for ft in range(FT):
                gated_dmas.append(
                    nc.sync.dma_start(x23[ft][:], xn23_d.ap()[ft]))
            for dma in gated_dmas:
                add_dep_helper(dma.ins, x1_dmas[-1].ins, sync=True,
                               reason="dma-priority")

            nc.vector.memset(ebias[:], EXP_BIAS)
            # warm the ACT exp table while DMAs run
            warm = const.tile([128, 1], f32, tag="warm", bufs=1, name="warm")
            nc.scalar.activation(warm[:], ebias[:], AF.Exp)

            # ---- PSUM pools: scores A (4 banks) + B (2) + work (2x1) ----
            ps_a = ctx.enter_context(
                tc.tile_pool(name="ps_a", bufs=1, space="PSUM"))
            ps_b = ctx.enter_context(
                tc.tile_pool(name="ps_b", bufs=1, space="PSUM"))
            ps_w = ctx.enter_context(
                tc.tile_pool(name="ps_w", bufs=2, space="PSUM"))

            mm = nc.tensor.matmul

            # ---- work units ----
            def qk_proj_block(w_idx, bias_sb, dst, m, nb):
                w_sb = wm_sb[w_idx][m]
                ns = slice(nb * QBS, (nb + 1) * QBS)
                ps = ps_w.tile([128, QBS], f32, tag="w", name="pspj")
                last = None
                for ft in range(FT):
                    last = mm(
                       ps[:],
                       w_sb[:, ft * 128:(ft + 1) * 128],
                       xt(ft, nb),
                       start=(ft == 0), stop=(ft == FT - 1))
                nc.vector.tensor_scalar_add(dst[:, ns], ps[:], bias_sb[:])
                return last

            ghost_scr = const.tile([128, 1], f32, tag="gscr", bufs=1,
                                   name="ghost_scr")

            def ghost_block(nb):
                """PE ballast: a projection re-run whose result is unused.
                Keeps the tensor engine's HAM activity window busy during
                ACT-bound stretches so the PE clock stays at 2.4 GHz."""
                ps = ps_w.tile([128, QBS], f32, tag="w", name="ghost")
                for ft in range(FT):
                    mm(ps[:],
                       wm_sb[1][0][:, ft * 128:(ft + 1) * 128],
                       xt(ft, nb),
                       start=(ft == 0), stop=(ft == FT - 1))
                nc.vector.tensor_copy(ghost_scr[:], ps[:, 0:1])

            def v_proj_block(st):
                nb, within = divmod(st, 4)
                ws = slice(within * 128, (within + 1) * 128)
                ps = ps_w.tile([128, QBS], f32, tag="w", name="pspjv")
                for ft in range(FT):
                    mm(ps[:, 0:VW],
                       xt(ft, nb)[:, ws],
                       wv_sb[:, ft * VW:(ft + 1) * VW],
                       start=(ft == 0), stop=(ft == FT - 1))
                nc.vector.tensor_scalar_mul(
                    vones[st][:], ps[:, 0:VW], em_sb[:, st:st + 1])
                for hh in range(HPC):
                    c = hh * (HD + 1) + HD
                    nc.vector.tensor_copy(
                        vones[st][:, c:c + 1], em_sb[:, st:st + 1])

            def scores_batch(it, side, bi):
                hp, qb = divmod(it, QB)
                k0, nk, pool = BATCH_PAT[side][bi]
                p0 = side * 64
                qs = slice(qb * QBS, (qb + 1) * QBS)
                pl = (ps_a, ps_b)[pool]
                w = nk * QBS
                ps = pl.tile([128, (4, 2)[pool] * QBS], f32,
                             tag="sc", name=f"ps{'AB'[pool]}")
                for j in range(nk):
                    ktile = k0 + j
                    ks = slice(ktile * 128, (ktile + 1) * 128)
                    js = slice(j * QBS, (j + 1) * QBS)
                    mm(ps[:, js],
                       kt_sb[hp][p0:p0 + 64, ks], qt[hp][p0:p0 + 64, qs],
                       tile_position=(p0, 0))
                es = slice(k0 * QBS, k0 * QBS + w)
                nc.scalar.activation(esb[side][:, es], ps[:, 0:w],
                                     AF.Exp, bias=ebias[:], scale=0.125)

            def ctx_batch(it, side, bi):
                hp, _ = divmod(it, QB)
                hh = 2 * hp + side
                k0, nk, _ = BATCH_PAT[side][bi]
                e = esb[side]
                ps = ps_w.tile([128, QBS], f32, tag="w", name="pscx")
                for qq in range(4):
                    dst = ps[:, qq * (HD + 1):(qq + 1) * (HD + 1)]
                    for j in range(nk):
                        ktile = k0 + j
                        lo = ktile * QBS + qq * 128
                        mm(dst,
                           e[:, lo:lo + 128],
                           vones[ktile][:, hh * (HD + 1):(hh + 1) * (HD + 1)],
                           start=(j == 0), stop=(j == nk - 1))
                if bi == 0:
                    nc.vector.tensor_copy(acc[side][:], ps[:, 0:4 * (HD + 1)])
                else:
                    nc.vector.tensor_add(acc[side][:], acc[side][:],
                                         ps[:, 0:4 * (HD + 1)])

            def ctx_final(it, side, ots):
                hp, qb = divmod(it, QB)
                hh = 2 * hp + side
                for qq in range(4):
                    if side == 0:
                        ot = opool.tile([128, 128], f32, tag="ot", bufs=8,
                                        name="ot")
                        ots.append(ot)
                    else:
                        ot = ots[qq]
                    r = rpool.tile([128, 1], f32, tag="r", bufs=8, name="r")
                    a = acc[side]
                    nc.vector.reciprocal(
                        r[:], a[:, qq * (HD + 1) + HD: qq * (HD + 1) + HD + 1])
                    nc.vector.scalar_tensor_tensor(
                        ot[:, side * 64:(side + 1) * 64],
                        a[:, qq * (HD + 1): qq * (HD + 1) + HD], r[:],
                        bvb_sb[:, hh * HD:(hh + 1) * HD],
                        op0=OP.mult, op1=OP.add)
                    if side == 1:
                        qt_idx = qb * 4 + qq
                        nc.sync.dma_start(
                            out_d.ap()[qt_idx * 128:(qt_idx + 1) * 128,
                                       hp * 128:(hp + 1) * 128],
                            ot[:])

            # ---- emission schedule ----
            # Slot atoms per iteration: side A's 5 scores batches, ctx
            # lagging 3 batches; side A's trailing ctx batches spread into
            # side B's scores; side B's trailing ctx + final carry into
            # the next iteration's start (so every PE wait point has
            # independent work queued behind it and no gap grows past
            # ~1us — the HAM clock gate re-throttles on longer idles):
            #  0:(s,A,0) 1:(s,A,1) 2:(s,A,2) 3:(s,A,3) 4:(c,A,0) 5:(s,A,4)
            #  6:(c,A,1) 7:(s,B,0) 8:(c,A,2) 9:(s,B,1) 10:(c,A,3) 11:(s,B,2)
            #  12:(c,A,4) 13:(F,A) 14:(s,B,3) 15:(c,B,0) 16:(s,B,4) 17:(c,B,1)
            def K0(nb):
                return lambda: qk_proj_block(0, bk_sb[0], kt_sb[0], 0, nb)

            def K1(nb):
                return lambda: qk_proj_block(0, bk_sb[1], kt_sb[1], 1, nb)

            def Q0(nb):
                return lambda: qk_proj_block(1, bq_sb[0], qt[0], 0, nb)

            def Q1(nb):
                return lambda: qk_proj_block(1, bq_sb[1], qt[1], 1, nb)

            def V(st):
                return lambda: v_proj_block(st)

            # fillers[it] = list of (atom_idx, thunk): emit before that atom.
            # Placement honors DMA arrival order (wk, wq, x01, wv, x23) and
            # consumer deadlines (scores need kt/qt; ctx batch bi reads
            # vones[k0..k0+nk-1]; it N+1 reads carry ctx of it N).
            def G(nb):
                return lambda: ghost_block(nb)

            fillers = {
                0: [(1, K0(1)),
                    (2, V(0)), (2, V(1)), (2, V(2)), (2, V(3)),
                    (2, K1(0)), (2, Q1(0)), (2, K0(2)),
                    (3, V(4)), (3, V(5)),
                    (5, K0(3)),
                    (7, V(6)), (7, V(7)), (7, V(8)), (7, V(9)),
                    (9, V(10)), (9, V(11)),
                    (11, V(12)), (11, V(13)), (13, V(14)), (13, V(15)),
                    (21, Q0(1))],
                1: [(0, Q0(2)), (9, K1(1)), (16, G(0))],
                2: [(0, Q0(3)), (9, K1(2)), (16, G(1))],
                3: [(0, K1(3)), (9, Q1(1)), (16, G(2))],
                4: [(0, Q1(2)), (9, G(3)), (16, G(0))],
                5: [(0, Q1(3)), (9, G(1)), (16, G(2))],
                6: [(0, G(3)), (9, G(0)), (16, G(1))],
                7: [(0, G(2)), (9, G(3)), (16, G(0))],
            }

            # head: minimal work to unblock it0's first scores batch
            qk_proj_block(0, bk_sb[0], kt_sb[0], 0, 0)
            qk_proj_block(1, bq_sb[0], qt[0], 0, 0)

            # Atom stream per iteration: side A's 6 scores batches with its
            # first 3 ctx batches inline (lag 3); side A's trailing ctx
            # spread into side B's 6 scores batches; side B's last 3 ctx
            # batches + final carry into the next iteration.
            #  0:sA0 1:sA1 2:sA2 3:sA3 4:cA0 5:sA4 6:cA1 7:sA5 8:cA2
            #  9:sB0 10:cA3 11:sB1 12:cA4 13:sB2 14:cA5 15:FA
            #  16:sB3 17:cB0 18:sB4 19:cB1 20:sB5 21:cB2
            atoms = []
            for bi in range(6):
                atoms.append(("s", 0, bi))
                if bi >= 3:
                    atoms.append(("c", 0, bi - 3))
            for bi in range(6):
                atoms.append(("s", 1, bi))
                if bi <= 2:
                    atoms.append(("c", 0, bi + 3))
                    if bi == 2:
                        atoms.append(("F", 0, 0))
                else:
                    atoms.append(("c", 1, bi - 3))

            carry = []  # side B trailing work, emitted at next it's start
            ots_map = {}
            for it in range(8):
                ots_map[it] = []
                flist = list(fillers.get(it, []))
                # position-0 fillers go FIRST: the PE queue is in-order, so
                # the carry ctx (waiting on the previous iteration's last
                # exps) must not block independent filler work behind it.
                while flist and flist[0][0] <= 0:
                    flist.pop(0)[1]()
                for c in carry[:2]:
                    c()
                cq = list(carry[2:])
                for idx, atom in enumerate(atoms):
                    while flist and flist[0][0] <= idx:
                        flist.pop(0)[1]()
                    kind, side, bi = atom
                    if kind == "s":
                        scores_batch(it, side, bi)
                    elif kind == "c":
                        ctx_batch(it, side, bi)
                    else:
                        ctx_final(it, 0, ots_map[it])
                    if cq:
                        cq.pop(0)()
                for f in flist:
                    f[1]()
                for c in cq:
                    c()
                carry = [
                    (lambda it=it: ctx_batch(it, 1, 3)),
                    (lambda it=it: ctx_batch(it, 1, 4)),
                    (lambda it=it: ctx_batch(it, 1, 5)),
                    (lambda it=it: ctx_final(it, 1, ots_map[it])),
                ]
            for c in carry:
                c()

    return nc


def _get_program(split_waits=True):
    key = ("nc", split_waits)
    if key not in _CACHE:
        _CACHE[key] = _build_program(split_waits)
    return _CACHE[key]


def _make_in_maps(hidden_states, attention_mask, Wq, bq, Wk, bk, Wv, bv):
    hidden = np.ascontiguousarray(np.asarray(hidden_states, dtype=np.float32))
    mask = np.asarray(attention_mask, dtype=np.float32)
    Wq = np.asarray(Wq, dtype=np.float32)
    Wk = np.asarray(Wk, dtype=np.float32)
    Wv = np.asarray(Wv, dtype=np.float32)
    bq = np.asarray(bq, dtype=np.float32)
    bk = np.asarray(bk, dtype=np.float32)
    bv = np.asarray(bv, dtype=np.float32)

    WqT = Wq.T  # [in, out]
    WkT = Wk.T
    WvT = Wv.T

    def pack_wm(WT, cols, m):
        # [H, 128] (m-half) -> [128, 8*128]: per f-tile 128-col blocks
        w = WT[:, cols][:, m * 128:(m + 1) * 128].astype(np.float16)
        return np.ascontiguousarray(
            w.reshape(FT, 128, 128).transpose(1, 0, 2).reshape(128, FT * 128))

    in_maps = []
    for c in range(NCORES):
        b, hg = divmod(c, HPC)
        cols = slice(hg * DS, (hg + 1) * DS)
        xT = hidden[b].T.astype(np.float16)  # [1024, 2048]
        xn0 = np.ascontiguousarray(xT[:, 0:512].reshape(FT, 128, 512))
        xn1 = np.ascontiguousarray(xT[:, 512:1024].reshape(FT, 128, 512))
        xn23 = np.ascontiguousarray(
            xT[:, 1024:2048].reshape(FT, 128, 1024))
        wv_base = WvT[:, cols].astype(np.float16)  # [1024, 256]
        wvA = np.zeros((128, FT * VW), np.float16)
        for ft in range(FT):
            blk = wv_base[ft * 128:(ft + 1) * 128]  # [128, 256]
            for hh in range(HPC):
                wvA[:, ft * VW + hh * (HD + 1): ft * VW + hh * (HD + 1) + HD] \
                    = blk[:, hh * HD:(hh + 1) * HD]
        bq_c = np.ascontiguousarray(bq[cols].reshape(2, 128, 1))
        bk_c = np.ascontiguousarray(bk[cols].reshape(2, 128, 1))
        bvb = np.ascontiguousarray(np.tile(bv[cols][None, :], (128, 1)))
        em = np.ascontiguousarray(
            np.exp(mask[b, 0, 0, :]).reshape(KT, 128).T.astype(np.float32))
        in_maps.append({
            "wk0": pack_wm(WkT, cols, 0), "wk1": pack_wm(WkT, cols, 1),
            "wq0": pack_wm(WqT, cols, 0), "wq1": pack_wm(WqT, cols, 1),
            "wvA": wvA,
            "xn0": xn0, "xn1": xn1, "xn23": xn23,
            "bq": bq_c, "bk": bk_c, "bvb": bvb, "em": em,
        })
    return in_maps


def _assemble(results):
    out = np.empty((B, S, H), np.float32)
    for c in range(NCORES):
        b, hg = divmod(c, HPC)
        out[b][:, hg * DS:(hg + 1) * DS] = results[c]["out"]
    return out


def _run(in_maps, trace=False):
    from concourse.bass_utils import run_bass_kernel_spmd
    nc = _get_program()
    return run_bass_kernel_spmd(
        nc, in_maps, core_ids=list(range(NCORES)), trace=trace)


def kernel(**inputs):
    in_maps = _make_in_maps(**inputs)
    res = _run(in_maps, trace=False)
    out = _assemble(res.results)
    if not np.isfinite(out).all():
        # rare device flakiness: retry once (NEFF is already compiled)
        res = _run(in_maps, trace=False)
        out = _assemble(res.results)
    return out


# revision 47
# speedup vs baseline: 1.0791x; 1.0791x over previous
"""BertSelfAttention Trainium2 kernel.

Full inputs in, full output out. Sharding: 8 cores = (batch b in {0,1}) x
(head-group hg in {0..3}); each core computes 4 heads of one batch and
produces the output feature slice out[b, :, hg*256:(hg+1)*256].

Per-core device program (all cores run the same NEFF, SPMD):
  xT [1024, 2048]      hidden_states[b].T
  QT/KT computed transposed [d, s] (fp32r matmuls), stored fp16 with bias
  V computed [s, d] fp16, rows scaled by exp(mask), plus a per-head
    ones*exp(mask) column so the ctx matmul also yields softmax row sums
  scoresT [k, q] tiles via fp16 matmuls (two heads on the two PE
    row-halves via tile_position)
  exp on ACT directly from PSUM (scale=1/8, bias=-4 folded in)
  ctx[q, d] = expT.T @ [V|em] accumulated over 16 k-tiles, then
    per-partition normalize (reciprocal of row sum) + V-bias add on DVE.

All PE instructions are chained with nosync deps in a hand-balanced
order (scores batches / ctx groups / projection filler) so the PE never
idles long enough for the HAM clock gate to re-throttle it to 1.2 GHz.
"""

import numpy as np

B = 2
S = 2048
H = 1024
NH = 16
HD = 64

NCORES = 8
HPC = 4          # heads per core
DS = HPC * HD    # 256 output dims per core
FT = H // 128    # 8 f-tiles (contraction tiles for projections)
KT = S // 128    # 16 key tiles
ST = S // 128    # 16 s-tiles of V
QB = 4           # q blocks of 512
QBS = 512
VW = HPC * (HD + 1)  # 260: V columns + one em column per head

EXP_BIAS = -4.0  # uniform shift inside exp; cancels in softmax, guards fp16

_CACHE = {}


def _build_program(split_waits=True):
    import concourse.bass as bass
    import concourse.mybir as mybir
    import concourse.tile as tile
    from concourse.tile_rust import add_dep_helper
    from concourse.vector_clock import ScopedClock

    f32 = mybir.dt.float32
    f32r = mybir.dt.float32r
    f16 = mybir.dt.float16
    AF = mybir.ActivationFunctionType
    OP = mybir.AluOpType

    class SplitDrainTileContext(tile.TileContext):
        """The walrus build here rejects instructions with more than one
        sync wait ("Too many sync wait commands"); hoist excess waits onto
        preceding same-engine NOPs."""

        MAX_WAITS_PER_DRAIN = 1
        split_waits_enabled = True

        def _drain_and_barrier(self, tick_clock, wait_clock):
            drain_inst = self.nc.sync.drain()
            wait_clock.add_sem_waits(
                drain_inst.ins, ScopedClock({None: tick_clock.global_clock})
            )
            self.nc.all_engine_barrier()
            assert self.sems is not None
            popped = self.nc._tile_sem_poison_stack.pop()
            assert popped is self._sem_poison
            self.nc.clear_and_free_semaphores(list(self.sems.allocated().values()))
            self.nc.all_engine_barrier()
            if self.split_waits_enabled:
                self._split_multi_waits()

        def _split_multi_waits(self):
            k = self.MAX_WAITS_PER_DRAIN
            nc = self.nc
            for bb in nc.bb_map.values():
                il = bb.bb.instructions
                new = []
                for inst in il:
                    si = getattr(inst, "sync_info", None)
                    waits = list(si.on_wait) if si is not None and si.on_wait else []
                    if len(waits) > k:
                        for j in range(0, len(waits) - k, k):
                            nop = mybir.InstNoOp(
                                name=nc.get_next_instruction_name(),
                                engine=inst.engine,
                                sync_info=mybir.SyncInfo(
                                    on_wait=waits[j : j + k], on_update=[]
                                ),
                                bass_nofuse=True,
                            )
                            new.append(nop)
                        inst.sync_info = mybir.SyncInfo(
                            on_wait=waits[len(waits) - k :],
                            on_update=list(si.on_update) if si.on_update else [],
                        )
                    new.append(inst)
                il[:] = new

    nc = bass.Bass("TRN2", target_bir_lowering=False, debug=False,
                   num_devices=NCORES)

    xT_d = nc.dram_tensor("xT", [H, S], f16, kind="ExternalInput")
    wqT_d = nc.dram_tensor("wqT", [H, DS], f16, kind="ExternalInput")
    wkT_d = nc.dram_tensor("wkT", [H, DS], f16, kind="ExternalInput")
    wvT_d = nc.dram_tensor("wvT", [H, VW], f16, kind="ExternalInput")
    bq_d = nc.dram_tensor("bq", [2, 128, 1], f32, kind="ExternalInput")
    bk_d = nc.dram_tensor("bk", [2, 128, 1], f32, kind="ExternalInput")
    bvb_d = nc.dram_tensor("bvb", [128, DS], f32, kind="ExternalInput")
    em_d = nc.dram_tensor("em", [128, KT], f32, kind="ExternalInput")
    out_d = nc.dram_tensor("out", [S, DS], f32, kind="ExternalOutput")

    SplitDrainTileContext.split_waits_enabled = split_waits
    with SplitDrainTileContext(nc) as tc:
        from contextlib import ExitStack

        with ExitStack() as ctx:
            const = ctx.enter_context(tc.tile_pool(name="const", bufs=1))
            qk = ctx.enter_context(tc.tile_pool(name="qk", bufs=1))
            vp = ctx.enter_context(tc.tile_pool(name="vp", bufs=1))
            epool = ctx.enter_context(tc.tile_pool(name="epool", bufs=1))
            opool = ctx.enter_context(tc.tile_pool(name="opool", bufs=1))
            rpool = ctx.enter_context(tc.tile_pool(name="rpool", bufs=1))

            # ---- constants ----
            bq_sb = [const.tile([128, 1], f32, tag=f"bq{m}", bufs=1,
                                name=f"bq_sb{m}") for m in range(2)]
            bk_sb = [const.tile([128, 1], f32, tag=f"bk{m}", bufs=1,
                                name=f"bk_sb{m}") for m in range(2)]
            for m in range(2):
                nc.sync.dma_start(bq_sb[m][:], bq_d.ap()[m])
                nc.sync.dma_start(bk_sb[m][:], bk_d.ap()[m])
            bvb_sb = const.tile([128, DS], f32, tag="bvb", bufs=1, name="bvb_sb")
            nc.sync.dma_start(bvb_sb[:], bvb_d.ap())
            em_sb = const.tile([128, KT], f32, tag="em", bufs=1, name="em_sb")
            nc.sync.dma_start(em_sb[:], em_d.ap())
            ebias = const.tile([128, 1], f32, tag="ebias", bufs=1, name="ebias")
            nc.vector.memset(ebias[:], EXP_BIAS)
            # warm the ACT exp table while DMAs run
            warm = const.tile([128, 1], f32, tag="warm", bufs=1, name="warm")
            nc.scalar.activation(warm[:], ebias[:], AF.Exp)

            # ---- persistent activations ----
            qt = [qk.tile([128, S], f16, tag=f"qt{m}", bufs=1, name=f"qt{m}")
                  for m in range(2)]
            kt_sb = [qk.tile([128, S], f16, tag=f"kt{m}", bufs=1, name=f"kt{m}")
                     for m in range(2)]
            vones = [vp.tile([128, VW], f16, tag=f"v{st}", bufs=1,
                             name=f"vones{st}") for st in range(ST)]

            # ---- input DMAs (xT + m0-needed weights first) ----
            xw = ctx.enter_context(tc.tile_pool(name="xw", bufs=1))
            xt = [[xw.tile([128, QBS], f16, tag=f"xt{ft}_{nb}", bufs=1,
                            name=f"xt{ft}_{nb}") for nb in range(QB)]
                  for ft in range(FT)]
            wq_sb = [xw.tile([128, DS], f16, tag=f"wq{ft}", bufs=1,
                             name=f"wq{ft}") for ft in range(FT)]
            wk_sb = [xw.tile([128, DS], f16, tag=f"wk{ft}", bufs=1,
                             name=f"wk{ft}") for ft in range(FT)]
            wv_sb = [xw.tile([128, VW], f16, tag=f"wv{ft}", bufs=1,
                             name=f"wv{ft}") for ft in range(FT)]
            # Tier 0: weights + first x block (unblocks the m0 projection
            # pipeline); later x blocks and wv are gated behind it so the
            # critical path gets full DMA bandwidth.
            tier0 = []
            for ft in range(FT):
                fs = slice(ft * 128, (ft + 1) * 128)
                tier0.append(nc.sync.dma_start(wq_sb[ft][:],
                                               wqT_d.ap()[fs, :]))
                tier0.append(nc.sync.dma_start(wk_sb[ft][:],
                                               wkT_d.ap()[fs, :]))
            for ft in range(FT):
                fs = slice(ft * 128, (ft + 1) * 128)
                tier0.append(nc.sync.dma_start(xt[ft][0][:],
                                               xT_d.ap()[fs, 0:QBS]))
            tier1 = []
            for nb in range(1, QB):
                ns = slice(nb * QBS, (nb + 1) * QBS)
                for ft in range(FT):
                    fs = slice(ft * 128, (ft + 1) * 128)
                    tier1.append(nc.sync.dma_start(xt[ft][nb][:],
                                                   xT_d.ap()[fs, ns]))
            for ft in range(FT):
                fs = slice(ft * 128, (ft + 1) * 128)
                tier1.append(nc.sync.dma_start(wv_sb[ft][:],
                                               wvT_d.ap()[fs, :]))
            for dma in tier1:
                add_dep_helper(dma.ins, tier0[-1].ins, sync=True,
                               reason="dma-priority")

            # ---- PSUM pools: proj 1 + scores 2x3 + ctx 1 = 8 banks ----
            ps_pj = ctx.enter_context(
                tc.tile_pool(name="ps_pj", bufs=1, space="PSUM"))
            ps_sc = ctx.enter_context(
                tc.tile_pool(name="ps_sc", bufs=2, space="PSUM"))
            ps_cx = ctx.enter_context(
                tc.tile_pool(name="ps_cx", bufs=1, space="PSUM"))

            mm = nc.tensor.matmul

            # ---- work units (each emits PE work + its evictions) ----
            def qk_proj_block(w_sb, bias_sb, dst, m, nb):
                ns = slice(nb * QBS, (nb + 1) * QBS)
                ps = ps_pj.tile([128, QBS], f32, tag="pj", name="pspj")
                for ft in range(FT):
                    mm(ps[:],
                       w_sb[ft][:, m * 128:(m + 1) * 128],
                       xt[ft][nb][:],
                       start=(ft == 0), stop=(ft == FT - 1))
                nc.vector.tensor_scalar_add(dst[:, ns], ps[:], bias_sb[:])

            def v_proj_block(st):
                nb, within = divmod(st, 4)
                ws = slice(within * 128, (within + 1) * 128)
                ps = ps_pj.tile([128, QBS], f32, tag="pj", name="pspjv")
                for ft in range(FT):
                    mm(ps[:, 0:VW],
                       xt[ft][nb][:, ws],
                       wv_sb[ft][:],
                       start=(ft == 0), stop=(ft == FT - 1))
                nc.vector.tensor_scalar_mul(
                    vones[st][:], ps[:, 0:VW], em_sb[:, st:st + 1])
                for hh in range(HPC):
                    c = hh * (HD + 1) + HD
                    nc.vector.tensor_copy(
                        vones[st][:, c:c + 1], em_sb[:, st:st + 1])

            BATCHES = [(0, 3), (3, 3), (6, 3), (9, 3), (12, 3), (15, 1)]

            def scores_batch(hp, qb, eA, eB, k0, nk, half=None):
                qs = slice(qb * QBS, (qb + 1) * QBS)
                w = nk * QBS
                es = slice(k0 * QBS, k0 * QBS + w)
                out = []
                for (side, e, p0) in ((0, eA, 0), (1, eB, 64)):
                    if half is not None and half != side:
                        continue
                    ps = ps_sc.tile([128, 3 * QBS], f32, tag="sc",
                                    name="pscA" if side == 0 else "pscB")
                    for j in range(nk):
                        ktile = k0 + j
                        ks = slice(ktile * 128, (ktile + 1) * 128)
                        js = slice(j * QBS, (j + 1) * QBS)
                        mm(ps[:, js],
                           kt_sb[hp][p0:p0 + 64, ks], qt[hp][p0:p0 + 64, qs],
                           tile_position=(p0, 0))
                    out.append(nc.scalar.activation(
                        eA[:, es] if side == 0 else eB[:, es], ps[:, 0:w],
                        AF.Exp, bias=ebias[:], scale=0.125))
                return out

            def ctx_unit(hp, qb, a, e, ots, delay_dep=None):
                """One head's 4 q-tiles of ctx for (hp, qb)."""
                hh = 2 * hp + a
                qtile0 = qb * 4
                cpsb = ps_cx.tile([128, 4 * (HD + 1)], f32, tag="cx",
                                  name="cps")
                for qq in range(4):
                    cps = cpsb[:, qq * (HD + 1):(qq + 1) * (HD + 1)]
                    for ktile in range(KT):
                        lo = ktile * QBS + qq * 128
                        inst = mm(cps,
                           e[:, lo:lo + 128],
                           vones[ktile][:, hh * (HD + 1):(hh + 1) * (HD + 1)],
                           start=(ktile == 0), stop=(ktile == KT - 1))
                        if delay_dep is not None and qq == 0 and ktile == 0:
                            add_dep_helper(inst.ins, delay_dep.ins, sync=True,
                                           reason="ctx-pacing")
                    if a == 0:
                        ot = opool.tile([128, 128], f32, tag="ot", bufs=4,
                                        name="ot")
                        ots.append(ot)
                    else:
                        ot = ots[qq]
                    r = rpool.tile([128, 1], f32, tag="r", bufs=4, name="r")
                    nc.vector.reciprocal(r[:], cps[:, HD:HD + 1])
                    nc.vector.scalar_tensor_tensor(
                        ot[:, a * 64:(a + 1) * 64],
                        cps[:, 0:HD], r[:],
                        bvb_sb[:, hh * HD:(hh + 1) * HD],
                        op0=OP.mult, op1=OP.add)
                    if a == 1:
                        qt_idx = qtile0 + qq
                        nc.sync.dma_start(
                            out_d.ap()[qt_idx * 128:(qt_idx + 1) * 128,
                                       hp * 128:(hp + 1) * 128],
                            ot[:])

            def ctx_units(prev_state, depA=None, depB=None):
                hp, qb, eA, eB = prev_state
                shared_ots = []
                return [
                    lambda: ctx_unit(hp, qb, 0, eA, shared_ots, delay_dep=depA),
                    lambda: ctx_unit(hp, qb, 1, eB, shared_ots, delay_dep=depB),
                ]

            # ---- emission schedule ----
            # m0 Q/K projection, pipelined by 512-col blocks
            for nb in range(QB):
                qk_proj_block(wq_sb, bq_sb[0], qt[0], 0, nb)
                qk_proj_block(wk_sb, bk_sb[0], kt_sb[0], 0, nb)

            # filler units per attention iteration index 0..7.
            # K-m1 must be done before iter 4 (all key columns are read by
            # every hp=1 iteration); Q-m1 for q-block X is first needed by
    
            # iteration 4+X, so those spread into the late iterations and
            # keep handing the PE a long uninterrupted accumulation chain
            # (~3.4us even cold) that re-arms the HAM warm window.
            def k1_block(nb):
                qk_proj_block(wk_sb, bk_sb[1], kt_sb[1], 1, nb)

            def q1_block(nb):
                qk_proj_block(wq_sb, bq_sb[1], qt[1], 1, nb)

            ghost_scr = const.tile([128, 1], f32, tag="gscr", bufs=1,
                                   name="ghost_scr")

            def ghost_block(nb):
                """PE ballast: projection re-run with unused result; keeps
                the HAM activity window busy through the tail so the PE
                clock is not re-throttled to 1.2 GHz."""
                ps = ps_pj.tile([128, QBS], f32, tag="pj", name="ghost")
                for ft in range(FT):
                    mm(ps[:],
                       wq_sb[ft][:, 0:128],
                       xt[ft][nb][:],
                       start=(ft == 0), stop=(ft == FT - 1))
                nc.vector.tensor_copy(ghost_scr[:], ps[:, 0:1])

            fillers = {
                0: [lambda st=st: v_proj_block(st) for st in range(8)],
                1: [lambda st=st: v_proj_block(st) for st in range(8, ST)],
                2: [lambda: k1_block(0), lambda: k1_block(1)],
                3: [lambda: k1_block(2), lambda: k1_block(3),
                    lambda: q1_block(0)],
                5: [lambda: q1_block(1)],
                6: [lambda: q1_block(2), lambda: ghost_block(0)],
                7: [lambda: q1_block(3), lambda: ghost_block(1)],
            }
            pre_fillers = {5, 6, 7}  # emit before the scores batches

            prev = None
            for it in range(8):
                hp, qb = divmod(it, QB)
                eA = epool.tile([128, KT * QBS], f16, tag="eA", bufs=3,
                                name="eA")
                eB = epool.tile([128, KT * QBS], f16, tag="eB", bufs=3,
                                name="eB")
                flist = fillers.get(it, [])
                if it in pre_fillers:
                    for filler in flist:
                        filler()
                    flist = []
                if it == 7:
                    # last iteration: all A-half batches first so eA
                    # completes early and the tail ctx overlaps the B exps
                    for (k0, nk) in BATCHES:
                        scores_batch(hp, qb, eA, eB, k0, nk, half=0)
                    ghost_block(2)
                    for (k0, nk) in BATCHES:
                        scores_batch(hp, qb, eA, eB, k0, nk, half=1)
                    ghost_block(3)
                else:
                    for (k0, nk) in BATCHES:
                        scores_batch(hp, qb, eA, eB, k0, nk)
                for filler in flist:
                    filler()
                if prev is not None:
                    for u in ctx_units(prev):
                        u()
                prev = (hp, qb, eA, eB)
            uA, uB = ctx_units(prev)
            uA()
            ghost_block(0)
            ghost_block(1)
            uB()

    return nc


def _get_program(split_waits=True):
    key = ("nc", split_waits)
    if key not in _CACHE:
        _CACHE[key] = _build_program(split_waits)
    return _CACHE[key]


def _make_in_maps(hidden_states, attention_mask, Wq, bq, Wk, bk, Wv, bv):
    hidden = np.ascontiguousarray(np.asarray(hidden_states, dtype=np.float32))
    mask = np.asarray(attention_mask, dtype=np.float32)
    Wq = np.asarray(Wq, dtype=np.float32)
    Wk = np.asarray(Wk, dtype=np.float32)
    Wv = np.asarray(Wv, dtype=np.float32)
    bq = np.asarray(bq, dtype=np.float32)
    bk = np.asarray(bk, dtype=np.float32)
    bv = np.asarray(bv, dtype=np.float32)

    WqT = Wq.T  # [in, out]
    WkT = Wk.T
    WvT = Wv.T

    in_maps = []
    for c in range(NCORES):
        b, hg = divmod(c, HPC)
        cols = slice(hg * DS, (hg + 1) * DS)
        xT = np.ascontiguousarray(hidden[b].T.astype(np.float16))
        wqT = np.ascontiguousarray(WqT[:, cols].astype(np.float16))
        wkT = np.ascontiguousarray(WkT[:, cols].astype(np.float16))
        wv_base = WvT[:, cols]
        wvT = np.zeros((H, VW), np.float16)
        for hh in range(HPC):
            wvT[:, hh * (HD + 1):hh * (HD + 1) + HD] = \
                wv_base[:, hh * HD:(hh + 1) * HD]
        bq_c = np.ascontiguousarray(bq[cols].reshape(2, 128, 1))
        bk_c = np.ascontiguousarray(bk[cols].reshape(2, 128, 1))
        bvb = np.ascontiguousarray(np.tile(bv[cols][None, :], (128, 1)))
        em = np.ascontiguousarray(
            np.exp(mask[b, 0, 0, :]).reshape(KT, 128).T.astype(np.float32))
        in_maps.append({
            "xT": xT, "wqT": wqT, "wkT": wkT, "wvT": wvT,
            "bq": bq_c, "bk": bk_c, "bvb": bvb, "em": em,
        })
    return in_maps


def _assemble(results):
    out = np.empty((B, S, H), np.float32)
    for c in range(NCORES):
        b, hg = divmod(c, HPC)
        out[b][:, hg * DS:(hg + 1) * DS] = results[c]["out"]
    return out


def _run(in_maps, trace=False):
    from concourse.bass_utils import run_bass_kernel_spmd
    nc = _get_program()
    return run_bass_kernel_spmd(
        nc, in_maps, core_ids=list(range(NCORES)), trace=trace)


def kernel(**inputs):
    in_maps = _make_in_maps(**inputs)
    res = _run(in_maps, trace=False)
    return _assemble(res.results)

